# revision 1
# baseline (speedup 1.0000x reference)
"""Trainium2 Bass kernel for the ChipletThermalModel problem.

Math per chiplet i, per grid point (summed over 16 chiplets):
  b± = B0 ± u, c± = C0 ± v   (u=(x-cx)/lx, v=(y-cy)/ly, B0=w/2lx, C0=h/2ly)
  For the 4 sign combos k=(s1,s2):
    S_k = a² + b² + c² ; δ_k = exp(½ lnS_k) ; 1/δ_k = exp(-½ lnS_k)
  Pair-merged log terms (two combos share one log):
    z12 = Σ_s1 b_s1·[ln((c_m+δ_s1m)(c_p+δ_s1p)) − ln(a²+b_s1²)]
        + Σ_s2 c_s2·[ln((b_m+δ_ms2)(b_p+δ_ps2)) − ln(a²+c_s2²)]
    At  = Σ_k atan(b c/(a δ_k))
  result += P_i·A·(2/√π)·(z12 − a·At) + A·B_off·ΣP_i

Engine split (per [128,1024] fp32 tile, per chiplet):
  DVE : 4 tensor_scalar (b±,c± from x,y: 594ns) + 23 STT ops (1127ns)
  Pool: 19 STT-form ops (1517ns)
  ACT : 26 activations (1038ns) in exactly 2 table phases per chiplet pair
        (ln+exp combined table, then arctan) -> 2 table loads per pair.
The activation-table view is patched in-place so the table-placement pass
picks natural_log_exp_and_others (contains BOTH ln and exp) instead of
ping-ponging between the ln-only and exp-only tables (which cost the
baseline 641 table loads = 822us of ACT time).

Sharding: batch dim (64) split across 8 cores -> 8 rows/core, laid out as
[128 partitions, 4096]; per-(batch-row,chiplet) parameters become
per-partition [128,1] scalars (host-precomputed).
"""
import sys
import numpy as np

for _p in ("/opt/trn_rl_repo",):
    if _p not in sys.path:
        sys.path.insert(0, _p)

N_CORES = 8
B, NCHIP, G2 = 64, 16, 65536
RPC = B // N_CORES            # batch rows per core = 8
P = 128                       # SBUF partitions
F = RPC * G2 // P             # free-dim columns per core = 4096
W = 1024                      # columns per processing group
NG = F // W                   # groups
WK_BUFS = 40                  # work-tile ring size
IN_PLACE = True               # reuse dead tiles for outputs
REP = P // RPC                # partitions per batch row = 16
NPAR = 6 * NCHIP + 1          # params columns (6 per chiplet + endC)
C1 = float(2.0 / np.sqrt(np.pi))


def _make_schedule(mode):
    """Emission order over (stage, chiplet). Engines dispatch in order with
    head-of-line blocking, so software-pipelining the emission (interleave
    the next chiplet's early stages with this chiplet's late stages) is
    what keeps all three engines fed."""
    s = []
    if mode == "pairs":
        for pair in range(NCHIP // 2):
            i0, i1 = 2 * pair, 2 * pair + 1
            s += [(k, i0) for k in range(5)]
            s += [(k, i1) for k in range(5)]
            s += [(5, i0), (5, i1)]
    elif mode == "sw15":
        # depth-1.5 pipeline: next chiplet's {st0,st1,st2} hoisted between
        # this chiplet's st2 and st3; atans batched per pair
        s = [(0, 0), (1, 0), (2, 0)]
        for i in range(NCHIP):
            s += [(0, i + 1), (3, i), (4, i), (1, i + 1), (2, i + 1)]
            if i % 2 == 1:
                s += [(5, i - 1), (5, i)]
    elif mode == "sw15i":
        # same but immediate atans (2 table loads per chiplet)
        s = [(0, 0), (1, 0), (2, 0)]
        for i in range(NCHIP):
            s += [(0, i + 1), (3, i), (4, i), (5, i), (1, i + 1), (2, i + 1)]
    elif mode == "pairs2":
        # ACT chain sees [sq+ln i0, sq+ln i1, lnp i0, lnp i1, at i0, at i1]:
        # while DVE computes the log-arg products of i0, ACT has i1's whole
        # ln block to chew on -> no idle before the lnp ops.
        for pair in range(NCHIP // 2):
            i0, i1 = 2 * pair, 2 * pair + 1
            s += [(0, i0), (1, i0), (0, i1), (1, i1),
                  (2, i0), (2, i1), (3, i0), (3, i1),
                  (4, i0), (4, i1), (5, i0), (5, i1)]
    elif mode.startswith("blk"):
        # generalized: process chiplets in blocks of K with phase-batched
        # ACT chain [sq+ln ×K, lnp ×K, at ×K]
        K = int(mode[3:])
        for b0 in range(0, NCHIP, K):
            blk = list(range(b0, min(b0 + K, NCHIP)))
            for i in blk:
                s += [(0, i), (1, i)]
            for i in blk:
                s += [(2, i)]
            for i in blk:
                s += [(3, i)]
            for i in blk:
                s += [(4, i)]
            for i in blk:
                s += [(5, i)]
    return [t for t in s if t[1] < NCHIP]


SCHEDULE = _make_schedule("pairs")


def _patch_activation_tables(nc):
    """Narrow the table view so ln/exp both resolve to the combined
    natural_log_exp_and_others table. get_activation_tables is cached and
    returns a shared dict -> in-place set mutation affects the placement
    pass and the simulators alike. Table *indices* are unchanged, so the
    emitted act_func_set_ids still match the compiler's act_info.json."""
    import concourse.mybir as mybir
    from concourse.hw_specs import get_activation_tables

    AF = mybir.ActivationFunctionType
    tabs = get_activation_tables(nc.m.arch)
    for name, funcs in tabs.items():
        if name != "natural_log_exp_and_others":
            funcs.discard(AF.Ln)
            funcs.discard(AF.Exp)


def _build_program(scal):
    """Build the Bass program. `scal` holds python-float per-chiplet scalars."""
    from concourse import bacc, tile
    import concourse.mybir as mybir
    import bass_rust as _bass_rust

    AF = mybir.ActivationFunctionType
    OP = mybir.AluOpType
    FP32 = mybir.dt.float32

    nc = bacc.Bacc("TRN2", target_bir_lowering=False, debug=False,
                   enable_asserts=False)
    _patch_activation_tables(nc)

    # Pin the ACT instruction order with scheduler-only (nosync) dep edges:
    # the engine is in-order so same-engine ordering costs nothing at
    # runtime, but it stops the list scheduler from interleaving the
    # ln/exp-table ops with arctan-table ops (table thrash).
    _act_prev = [None]

    def _act(out, in_, func, **kw):
        inst = nc.scalar.activation(out, in_, func, **kw)
        if _act_prev[0] is not None:
            _bass_rust.add_dep_helper(inst.ins, _act_prev[0], sync=False,
                                      reason="act table phase order")
        _act_prev[0] = inst.ins
        return inst

    xin = nc.dram_tensor("xin", [P, F], FP32, kind="ExternalInput")
    yin = nc.dram_tensor("yin", [P, F], FP32, kind="ExternalInput")
    prm = nc.dram_tensor("prm", [P, NPAR], FP32, kind="ExternalInput")
    eye = nc.dram_tensor("eye", [P, P], FP32, kind="ExternalInput")
    out = nc.dram_tensor("out", [P, F], FP32, kind="ExternalOutput")

    a2 = scal["a2"]
    inv_a = scal["inv_a"]
    inv_lx = scal["inv_lx"]
    inv_ly = scal["inv_ly"]

    MP = ("m", "p")
    HALF = W // 2

    with tile.TileContext(nc) as tc:
        with tc.tile_pool(name="cst", bufs=1) as cst, \
             tc.tile_pool(name="io", bufs=2) as io, \
             tc.tile_pool(name="ps", bufs=2, space="PSUM") as ps, \
             tc.tile_pool(name="wk", bufs=WK_BUFS) as wk:
            prmt = cst.tile([P, NPAR], FP32)
            nc.sync.dma_start(prmt[:], prm[:])
            eyet = cst.tile([P, P], FP32)
            nc.sync.dma_start(eyet[:], eye[:])

            def pcol(i, k):           # [128,1] per-partition param AP
                return prmt[:, 6 * i + k: 6 * i + k + 1]

            endC = prmt[:, 6 * NCHIP: 6 * NCHIP + 1]

            for g in range(NG):
                cs = slice(g * W, (g + 1) * W)
                xt = io.tile([P, W], FP32, tag="xt")
                yt = io.tile([P, W], FP32, tag="yt")
                res = io.tile([P, W], FP32, tag="res")
                nc.sync.dma_start(xt[:], xin[:, cs])
                nc.sync.dma_start(yt[:], yin[:, cs])
                # PSUM accumulators: one bank per half-tile (matmul moving
                # free dim is capped at 512)
                acc = [ps.tile([P, HALF], FP32, tag=f"acc{h}",
                               name=f"acc{h}")
                       for h in range(W // HALF)]
                mm_count = [0]
                MM_TOTAL = NCHIP * 8 * (W // HALF)

                def accum(t):
                    """res_psum += t via identity matmul on the idle PE."""
                    for h, a_ in enumerate(acc):
                        first = mm_count[0] < len(acc)
                        last = mm_count[0] >= MM_TOTAL - len(acc)
                        nc.tensor.matmul(
                            a_[:], eyet[:], t[:, h * HALF:(h + 1) * HALF],
                            start=first, stop=last)
                        mm_count[0] += 1

                def wtile(nm):
                    return wk.tile([P, W], FP32, tag="wk", name=nm)

                def reuse(old, nm):
                    """Output tile: reuse a dead tile's buffer (cuts ring
                    pressure) or allocate fresh when IN_PLACE is off."""
                    return old if IN_PLACE else wtile(nm)

                # per-chiplet state dicts, keyed by chiplet index
                st = [dict() for _ in range(NCHIP)]

                def st0(i):
                    """b±,c± (DVE TSP), squares, b²+c² (Pool)."""
                    e = st[i]
                    bs, cs_ = {}, {}
                    for k, sgn, col in (("m", -1.0, 0), ("p", 1.0, 1)):
                        t = wtile("b" + k)
                        nc.vector.tensor_scalar(t[:], xt[:], sgn * inv_lx[i],
                                                pcol(i, col), OP.mult, OP.add)
                        bs[k] = t
                    for k, sgn, col in (("m", -1.0, 2), ("p", 1.0, 3)):
                        t = wtile("c" + k)
                        nc.vector.tensor_scalar(t[:], yt[:], sgn * inv_ly[i],
                                                pcol(i, col), OP.mult, OP.add)
                        cs_[k] = t
                    sqb, sqc = {}, {}
                    for k in MP:
                        t = wtile("sqb")
                        _act(t[:], bs[k][:], AF.Square)
                        sqb[k] = t
                    t = wtile("sqc")
                    nc.gpsimd.tensor_tensor(t[:], cs_["m"][:], cs_["m"][:],
                                            OP.mult)
                    sqc["m"] = t
                    t = wtile("sqc")
                    nc.vector.scalar_tensor_tensor(t[:], cs_["p"][:], 1.0,
                                                   cs_["p"][:], OP.mult,
                                                   OP.mult)
                    sqc["p"] = t
                    s0 = {}
                    for kx in MP:
                        for ky in MP:
                            t = wtile("s0")
                            nc.gpsimd.tensor_tensor(
                                t[:], sqb[kx][:], sqc[ky][:], OP.add)
                            s0[kx + ky] = t
                    e.update(b=bs, c=cs_, sqb=sqb, sqc=sqc, s0=s0)

                def st1(i):
                    """ACT ln/exp block (single combined table, a² via bias)."""
                    e = st[i]
                    lnS, dl, rd = {}, {}, {}
                    for kk in e["s0"]:
                        t = wtile("lnS")
                        _act(t[:], e["s0"][kk][:], AF.Ln, bias=a2)
                        lnS[kk] = t
                    for kk in lnS:
                        t = wtile("dl")
                        _act(t[:], lnS[kk][:], AF.Exp, scale=0.5)
                        dl[kk] = t
                        t = reuse(lnS[kk], "rd")   # lnS dead after 2nd exp
                        _act(t[:], lnS[kk][:], AF.Exp, scale=-0.5)
                        rd[kk] = t
                    lax = {}
                    for side, sq in (("b", e["sqb"]), ("c", e["sqc"])):
                        for k in MP:
                            t = wtile("lax")
                            _act(t[:], sq[k][:], AF.Ln, bias=a2)
                            lax[side + k] = t
                    e.update(dl=dl, rd=rd, lax=lax)

                def st2(i):
                    """b+δ, c+δ, log-arg products (DVE); atan args early."""
                    e = st[i]
                    bs, cs_, dl = e["b"], e["c"], e["dl"]
                    cpd, bpd = {}, {}
                    for kx in MP:
                        for ky in MP:
                            kk = kx + ky
                            t = wtile("bpd")
                            nc.vector.scalar_tensor_tensor(
                                t[:], dl[kk][:], 0.0, bs[kx][:],
                                OP.add, OP.add)
                            bpd[kk] = t
                            t = reuse(dl[kk], "cpd")
                            nc.vector.scalar_tensor_tensor(
                                t[:], dl[kk][:], 0.0, cs_[ky][:],
                                OP.add, OP.add)
                            cpd[kk] = t
                    prod = {}
                    for kx in MP:      # b-side: fixed s1, product over s2
                        t = reuse(cpd[kx + "m"], "prod")
                        nc.vector.scalar_tensor_tensor(
                            t[:], cpd[kx + "m"][:], 1.0, cpd[kx + "p"][:],
                            OP.mult, OP.mult)
                        prod["b" + kx] = t
                    for ky in MP:      # c-side: fixed s2, product over s1
                        t = reuse(bpd["m" + ky], "prod")
                        nc.vector.scalar_tensor_tensor(
                            t[:], bpd["m" + ky][:], 1.0, bpd["p" + ky][:],
                            OP.mult, OP.mult)
                        prod["c" + ky] = t
                    # atan args early (need only b,c,rd): by the time the
                    # ACT chain reaches the arctan block, inputs are ready
                    targ = {}
                    for kx in MP:
                        for ky in MP:
                            kk = kx + ky
                            t = wtile("bc")
                            nc.gpsimd.tensor_tensor(
                                t[:], bs[kx][:], cs_[ky][:], OP.mult)
                            tt = reuse(t, "targ")
                            nc.vector.scalar_tensor_tensor(
                                tt[:], t[:], inv_a, e["rd"][kk][:],
                                OP.mult, OP.mult)
                            targ[kk] = tt
                    e.update(prod=prod, targ=targ)

                def st3(i):
                    """One ln per side (ACT)."""
                    e = st[i]
                    lnp = {}
                    for sk in e["prod"]:
                        t = wtile("lnp")
                        _act(t[:], e["prod"][sk][:], AF.Ln)
                        lnp[sk] = t
                    e.update(lnp=lnp)

                def st4(i):
                    """q = PiA·b·(lnp − lax), accumulated into PSUM by PE."""
                    e = st[i]
                    bs, cs_, lax, lnp = e["b"], e["c"], e["lax"], e["lnp"]
                    for sk, mult in (("bm", bs["m"]), ("bp", bs["p"]),
                                     ("cm", cs_["m"]), ("cp", cs_["p"])):
                        t = reuse(lnp[sk], "d")
                        nc.gpsimd.tensor_tensor(
                            t[:], lnp[sk][:], lax[sk][:], OP.subtract)
                        tq = reuse(t, "q")
                        nc.vector.scalar_tensor_tensor(
                            tq[:], t[:], pcol(i, 4), mult[:],
                            OP.mult, OP.mult)
                        accum(tq)

                def st5(i):
                    """Arctan phase (other ACT table); −a·PiA·at into PSUM."""
                    e = st[i]
                    for kk, t in e["targ"].items():
                        ta = reuse(e["s0"][kk], "at")   # s0 dead after lnS
                        _act(ta[:], t[:], AF.Arctan)
                        tas = reuse(ta, "ats")
                        nc.vector.tensor_scalar(tas[:], ta[:], pcol(i, 5),
                                                None, OP.mult)
                        accum(tas)
                    st[i] = {}   # drop tile refs

                stages = [st0, st1, st2, st3, st4, st5]
                for step, i in SCHEDULE:
                    if 0 <= i < NCHIP:
                        stages[step](i)
                # evict PSUM -> SBUF (+endC), then DMA out
                for h in range(W // HALF):
                    nc.vector.tensor_scalar(
                        res[:, h * HALF:(h + 1) * HALF], acc[h][:], 1.0,
                        endC, OP.mult, OP.add)
                nc.sync.dma_start(out[:, cs], res[:])
    nc.finalize()
    return nc


def _host_params(cx, cy, w, h, Pw, A, a, B_off, lx, ly, rows):
    """Per-core [128, NPAR] parameter matrix (per-partition scalars)."""
    pr = np.zeros((P, NPAR), dtype=np.float32)
    for i in range(NCHIP):
        B0 = 0.5 * w[rows, i] / lx[i]
        C0 = 0.5 * h[rows, i] / ly[i]
        cxl = cx[rows, i] / lx[i]
        cyl = cy[rows, i] / ly[i]
        pr[:, 6 * i + 0] = np.repeat(B0 + cxl, REP)   # bm = −x/lx + (B0+cxl)
        pr[:, 6 * i + 1] = np.repeat(B0 - cxl, REP)   # bp = +x/lx + (B0−cxl)
        pr[:, 6 * i + 2] = np.repeat(C0 + cyl, REP)
        pr[:, 6 * i + 3] = np.repeat(C0 - cyl, REP)
        pr[:, 6 * i + 4] = np.repeat(Pw[rows, i] * A * C1, REP)
        pr[:, 6 * i + 5] = np.repeat(-a * Pw[rows, i] * A * C1, REP)
    pr[:, 6 * NCHIP] = np.repeat(A * B_off * Pw[rows].sum(axis=1), REP)
    return np.ascontiguousarray(pr, dtype=np.float32)


_CACHE = {}


def _get_executor(scal):
    """Build (once) the Bass program and a cached jitted SPMD callable.

    The callable takes (xin, yin, prm, scratch) as [8·128, ...] arrays
    sharded over 8 cores; scratch is a donated fp32 [8·128, F] buffer the
    kernel output aliases (every element is overwritten)."""
    if "exec" in _CACHE:
        return _CACHE["exec"]

    import jax
    import jax.numpy as jnp
    from jax.sharding import Mesh, NamedSharding, PartitionSpec
    from jax.experimental.shard_map import shard_map
    from concourse import bass2jax
    import concourse.mybir as mybir

    nc = _build_program(scal)
    _CACHE["nc"] = nc
    bass2jax.install_neuronx_cc_hook()

    partition_name = (nc.partition_id_tensor.name
                      if nc.partition_id_tensor else None)
    in_names, out_names, out_avals = [], [], []
    for alloc in nc.m.functions[0].allocations:
        if not isinstance(alloc, mybir.MemoryLocationSet):
            continue
        name = alloc.memorylocations[0].name
        if alloc.kind == "ExternalInput":
            if name != partition_name:
                in_names.append(name)
        elif alloc.kind == "ExternalOutput":
            out_names.append(name)
            out_avals.append(jax.core.ShapedArray(
                tuple(alloc.tensor_shape), mybir.dt.np(alloc.dtype)))
    n_params = len(in_names)
    all_names = in_names + out_names
    if partition_name is not None:
        all_names = all_names + [partition_name]

    def _body(*args):
        operands = list(args)
        if partition_name is not None:
            operands.append(bass2jax.partition_id_tensor())
        outs = bass2jax._bass_exec_p.bind(
            *operands,
            out_avals=tuple(out_avals),
            in_names=tuple(all_names),
            out_names=tuple(out_names),
            lowering_input_output_aliases=(),
            sim_require_finite=True,
            sim_require_nnan=True,
            nc=nc,
        )
        return tuple(outs)

    devices = jax.devices()[:N_CORES]
    mesh = Mesh(np.asarray(devices), ("core",))
    sharding = NamedSharding(mesh, PartitionSpec("core"))
    donate = tuple(range(n_params, n_params + len(out_avals)))
    sharded = jax.jit(
        shard_map(_body, mesh=mesh,
                  in_specs=(PartitionSpec("core"),) * (n_params + len(out_avals)),
                  out_specs=(PartitionSpec("core"),) * len(out_avals),
                  check_rep=False),
        donate_argnums=donate, keep_unused=True)

    # device-side scratch maker: avoids shipping 16MB of zeros per call
    zshapes = [(N_CORES * s.shape[0], *s.shape[1:]) for s in out_avals]
    zdtypes = [s.dtype for s in out_avals]

    def _mk(shape_dtype):
        shape, dtype = shape_dtype
        return jax.jit(lambda: jnp.zeros(shape, dtype),
                       out_shardings=sharding)

    zeros_fns = [_mk(sd) for sd in zip(zshapes, zdtypes)]
    ex = {"sharded": sharded, "in_names": in_names, "zeros_fns": zeros_fns,
          "sharding": sharding, "n_params": n_params}
    _CACHE["exec"] = ex
    return ex


def _scal_from_inputs(a, lx, ly):
    af = float(np.asarray(a).reshape(-1)[0])
    lxf = np.asarray(lx, dtype=np.float64)
    lyf = np.asarray(ly, dtype=np.float64)
    return {
        "a2": float(af * af),
        "inv_a": float(1.0 / af),
        "inv_lx": [float(1.0 / lxf[i]) for i in range(NCHIP)],
        "inv_ly": [float(1.0 / lyf[i]) for i in range(NCHIP)],
    }


def run(x, y, chiplets_x, chiplets_y, chiplets_width, chiplets_height,
        chiplets_power, A, a, B_off, lx, ly, grid=None):
    import jax

    x = np.asarray(x, dtype=np.float32)
    y = np.asarray(y, dtype=np.float32)
    cx = np.asarray(chiplets_x, dtype=np.float32)
    cy = np.asarray(chiplets_y, dtype=np.float32)
    w = np.asarray(chiplets_width, dtype=np.float32)
    h = np.asarray(chiplets_height, dtype=np.float32)
    Pw = np.asarray(chiplets_power, dtype=np.float32)
    Af = float(np.asarray(A).reshape(-1)[0])
    af = float(np.asarray(a).reshape(-1)[0])
    Bf = float(np.asarray(B_off).reshape(-1)[0])
    lxf = np.asarray(lx, dtype=np.float64)
    lyf = np.asarray(ly, dtype=np.float64)

    ex = _get_executor(_scal_from_inputs(a, lx, ly))

    # [8*128, F] stacked per-core blocks
    xs = np.ascontiguousarray(x.reshape(N_CORES * P, F))
    ys = np.ascontiguousarray(y.reshape(N_CORES * P, F))
    prs = np.concatenate(
        [_host_params(cx, cy, w, h, Pw, Af, af, Bf, lxf, lyf,
                      slice(c * RPC, (c + 1) * RPC)) for c in range(N_CORES)],
        axis=0)
    eye8 = np.ascontiguousarray(
        np.tile(np.eye(P, dtype=np.float32), (N_CORES, 1)))
    arrs = {"xin": xs, "yin": ys, "prm": prs, "eye": eye8}
    ins = [jax.device_put(arrs[nm], ex["sharding"]) for nm in ex["in_names"]]
    scratch = [zf() for zf in ex["zeros_fns"]]
    out = ex["sharded"](*ins, *scratch)
    full = np.asarray(out[0]).reshape(B, G2).astype(np.float32, copy=False)
    return full


def kernel(**inputs):
    return run(**inputs)



# revision 8
# speedup vs baseline: 50.3550x; 50.3550x over previous
"""Trainium2 Bass kernel for the ChipletThermalModel problem.

Math per chiplet i, per grid point (summed over 16 chiplets), after
normalizing by `a` (F(a,b,c) = a*F(1, b/a, c/a), so a^2 -> 1 and the
overall factor a folds into the per-chiplet scale):
  b'± = pb± ± x/(a*lx),  c'± = pc± ± y/(a*ly)       (pb,pc host-precomputed)
  For the 4 sign combos k=(s1,s2):  δ_k = sqrt(1 + b'² + c'²)
  b-side (pair-merged over s2):
    lnq_b(s1) = ln(1+b'²) - ln((c'm+δ_{s1m})(c'p+δ_{s1p}))
    contribution -= s_i * b'_{s1} * lnq_b(s1)        (s_i = Pi*A*a*2/√π)
  c-side symmetric; atan side: contribution -= s_i * atan(b'c' * (1/δ_k))
  (1/δ via the 1-instruction custom-DVE RECIPROCAL_APPROX_FAST, ~51 ULP;
  the HW ISA has no tensor-tensor divide) plus endC = A*B_off*ΣPi.

The -s_i scale is applied by the accumulating PE matmul with a diagonal
stationary matrix diag(-s_i[p]) (host-precomputed per partition), so no
elementwise scale ops remain.

Engine split per [128,1024] fp32 tile per chiplet:
  ACT : 16 ops in 3 table phases (4 Sqrt | 4+4 Ln | 4 Atan), phases are
        pair-blocked -> 3 table loads per chiplet pair; ln(1+b'²) uses the
        activation bias input so no squares/adds are spent on it.
  DVE : 4 prep tensor_scalar (594ns) + 4 recip + ~12 TT (1127ns)
  Pool: ~24 TT (853ns)
  PE  : 16 matmul-halves [128,512] fp32 accumulating into PSUM.

Sharding: batch dim (64) split across 8 cores -> 8 rows/core, laid out as
[128 partitions, 4096]; per-(batch-row,chiplet) parameters become
per-partition [128,1] scalars and [128,128] diag blocks (host-precomputed).
"""
import sys
import numpy as np

for _p in ("/opt/trn_rl_repo",):
    if _p not in sys.path:
        sys.path.insert(0, _p)

N_CORES = 8
B, NCHIP, G2 = 64, 16, 65536
RPC = B // N_CORES            # batch rows per core = 8
P = 128                       # SBUF partitions
F = RPC * G2 // P             # free-dim columns per core = 4096
W = 1024                      # columns per processing group
NG = F // W                   # groups
WK_BUFS = 40                  # work-tile ring size
REP = P // RPC                # partitions per batch row = 16
NPAR = 4 * NCHIP + 1          # params columns (4 per chiplet + endC)
C1 = float(2.0 / np.sqrt(np.pi))

# Which of the flexible elementwise ops run on DVE ("d") vs Pool ("p").
# pd is a list of 8 engine picks (4 cpd then 4 bpd).
ENG = {
    "sq": "d", "s0": "p",
    "pd": ["p", "p", "p", "p", "p", "p", "p", "p"],
    "prod": "p", "sub": "p", "bc": "d", "targ": "d", "blnq": "p",
}


def _make_schedule(mode):
    """Emission order over (stage, chiplet). Engines dispatch in order with
    head-of-line blocking; pair-blocked ACT phases keep table loads at 3
    per chiplet pair, and hoisting the next pair's early stages keeps the
    vector engines fed while ACT chews a phase."""
    s = []
    if mode == "pairs":
        for pr in range(NCHIP // 2):
            i0, i1 = 2 * pr, 2 * pr + 1
            s += [(0, i0), (1, i0), (0, i1), (1, i1),
                  (2, i0), (2, i1), (3, i0), (3, i1),
                  (4, i0), (4, i1), (5, i0), (5, i1),
                  (6, i0), (6, i1), (7, i0), (7, i1)]
    elif mode == "sw":
        # software-pipelined: next pair's prep/squares emitted between this
        # pair's ln phase and blnq stage
        s += [(0, 0), (1, 0), (0, 1), (1, 1)]
        for pr in range(NCHIP // 2):
            i0, i1 = 2 * pr, 2 * pr + 1
            j0, j1 = i0 + 2, i1 + 2
            s += [(2, i0), (2, i1), (3, i0), (3, i1),
                  (4, i0), (4, i1),
                  (0, j0), (1, j0), (0, j1), (1, j1),
                  (5, i0), (5, i1),
                  (6, i0), (6, i1), (7, i0), (7, i1)]
    return [t for t in s if 0 <= t[1] < NCHIP]


SCHEDULE = _make_schedule("sw")


def _build_program(scal):
    """Build the Bass program. `scal` holds python-float per-chiplet scalars."""
    from concourse import bacc, tile
    import concourse.mybir as mybir
    import bass_rust as _bass_rust

    AF = mybir.ActivationFunctionType
    OP = mybir.AluOpType
    FP32 = mybir.dt.float32

    nc = bacc.Bacc("TRN2", target_bir_lowering=False, debug=False,
                   enable_asserts=False)

    # Pin the ACT instruction order with scheduler-only (nosync) dep edges:
    # the engine is in-order so same-engine ordering costs nothing at
    # runtime, but it stops the list scheduler from interleaving ops of
    # different table phases (table thrash).
    _act_prev = [None]

    def _act(out, in_, func, **kw):
        inst = nc.scalar.activation(out, in_, func, **kw)
        if _act_prev[0] is not None:
            _bass_rust.add_dep_helper(inst.ins, _act_prev[0], sync=False,
                                      reason="act table phase order")
        _act_prev[0] = inst.ins
        return inst

    xin = nc.dram_tensor("xin", [P, F], FP32, kind="ExternalInput")
    yin = nc.dram_tensor("yin", [P, F], FP32, kind="ExternalInput")
    prm = nc.dram_tensor("prm", [P, NPAR], FP32, kind="ExternalInput")
    dgf = nc.dram_tensor("dgf", [P, NCHIP * P], FP32, kind="ExternalInput")
    out = nc.dram_tensor("out", [P, F], FP32, kind="ExternalOutput")

    inv_la = scal["inv_la"]
    inv_ha = scal["inv_ha"]

    MP = ("m", "p")
    HALF = W // 2

    def eng(which):
        return nc.vector if which == "d" else nc.gpsimd

    with tile.TileContext(nc) as tc:
        with tc.tile_pool(name="cst", bufs=1) as cst, \
             tc.tile_pool(name="io", bufs=2) as io, \
             tc.tile_pool(name="ps", bufs=2, space="PSUM") as ps, \
             tc.tile_pool(name="wk", bufs=WK_BUFS) as wk:
            prmt = cst.tile([P, NPAR], FP32)
            nc.sync.dma_start(prmt[:], prm[:])
            dgt = cst.tile([P, NCHIP * P], FP32)
            nc.sync.dma_start(dgt[:], dgf[:])

            def pcol(i, k):           # [128,1] per-partition param AP
                return prmt[:, 4 * i + k: 4 * i + k + 1]

            endC = prmt[:, 4 * NCHIP: 4 * NCHIP + 1]

            for g in range(NG):
                cs = slice(g * W, (g + 1) * W)
                xt = io.tile([P, W], FP32, tag="xt")
                yt = io.tile([P, W], FP32, tag="yt")
                res = io.tile([P, W], FP32, tag="res")
                nc.sync.dma_start(xt[:], xin[:, cs])
                nc.sync.dma_start(yt[:], yin[:, cs])
                # PSUM accumulators: one bank per half-tile (matmul moving
                # free dim is capped at 512)
                acc = [ps.tile([P, HALF], FP32, tag=f"acc{h}",
                               name=f"acc{h}")
                       for h in range(W // HALF)]
                mm_count = [0]
                MM_TOTAL = NCHIP * 8 * (W // HALF)

                def accum(t, i):
                    """res_psum += diag(-s_i) @ t on the PE."""
                    stat = dgt[:, i * P:(i + 1) * P]
                    for h, a_ in enumerate(acc):
                        first = mm_count[0] < len(acc)
                        last = mm_count[0] >= MM_TOTAL - len(acc)
                        nc.tensor.matmul(
                            a_[:], stat, t[:, h * HALF:(h + 1) * HALF],
                            start=first, stop=last)
                        mm_count[0] += 1

                def wtile(nm):
                    return wk.tile([P, W], FP32, tag="wk", name=nm)

                # per-chiplet state dicts, keyed by chiplet index
                st = [dict() for _ in range(NCHIP)]

                def st0(i):
                    """b'±, c'± via DVE tensor_scalar (x*(±1/(a*l)) + p)."""
                    e = st[i]
                    bs, cs_ = {}, {}
                    for k, sgn, col in (("m", -1.0, 0), ("p", 1.0, 1)):
                        t = wtile("b" + k)
                        nc.vector.tensor_scalar(t[:], xt[:], sgn * inv_la[i],
                                                pcol(i, col), OP.mult, OP.add)
                        bs[k] = t
                    for k, sgn, col in (("m", -1.0, 2), ("p", 1.0, 3)):
                        t = wtile("c" + k)
                        nc.vector.tensor_scalar(t[:], yt[:], sgn * inv_ha[i],
                                                pcol(i, col), OP.mult, OP.add)
                        cs_[k] = t
                    e.update(b=bs, c=cs_)

                def st1(i):
                    """squares and s0 = b'² + c'² per combo."""
                    e = st[i]
                    sqb, sqc = {}, {}
                    for k in MP:
                        t = wtile("sqb")
                        eng(ENG["sq"]).tensor_tensor(
                            t[:], e["b"][k][:], e["b"][k][:], OP.mult)
                        sqb[k] = t
                        t = wtile("sqc")
                        eng(ENG["sq"]).tensor_tensor(
                            t[:], e["c"][k][:], e["c"][k][:], OP.mult)
                        sqc[k] = t
                    s0 = {}
                    for kx in MP:
                        for ky in MP:
                            t = wtile("s0")
                            eng(ENG["s0"]).tensor_tensor(
                                t[:], sqb[kx][:], sqc[ky][:], OP.add)
                            s0[kx + ky] = t
                    e.update(sqb=sqb, sqc=sqc, s0=s0)

                def st2(i):
                    """ACT sqrt phase: δ = Sqrt(s0 + 1)."""
                    e = st[i]
                    dl = {}
                    for kk in ("mm", "mp", "pm", "pp"):
                        t = wtile("dl")
                        _act(t[:], e["s0"][kk][:], AF.Sqrt, bias=1.0)
                        dl[kk] = t
                    e.update(dl=dl)

                def st3(i):
                    """pd sums, pair products, 1/δ, bc, targ (DVE+Pool)."""
                    e = st[i]
                    bs, cs_, dl, s0 = e["b"], e["c"], e["dl"], e["s0"]
                    # cpd: c_s2 + δ (for b-side products); reuse dead s0 bufs
                    cpd, bpd = {}, {}
                    pdi = 0
                    for kx in MP:
                        for ky in MP:
                            kk = kx + ky
                            t = s0[kk]           # s0 dead after st2
                            eng(ENG["pd"][pdi]).tensor_tensor(
                                t[:], cs_[ky][:], dl[kk][:], OP.add)
                            cpd[kk] = t
                            pdi += 1
                    for kx in MP:
                        for ky in MP:
                            kk = kx + ky
                            t = wtile("bpd")
                            eng(ENG["pd"][pdi]).tensor_tensor(
                                t[:], bs[kx][:], dl[kk][:], OP.add)
                            bpd[kk] = t
                            pdi += 1
                    # pair products; write onto one of the inputs
                    prod = {}
                    for kx in MP:      # b-side: fixed s1, product over s2
                        t = cpd[kx + "m"]
                        eng(ENG["prod"]).tensor_tensor(
                            t[:], cpd[kx + "m"][:], cpd[kx + "p"][:],
                            OP.mult)
                        prod["b" + kx] = t
                    for ky in MP:      # c-side: fixed s2, product over s1
                        t = bpd["m" + ky]
                        eng(ENG["prod"]).tensor_tensor(
                            t[:], bpd["m" + ky][:], bpd["p" + ky][:],
                            OP.mult)
                        prod["c" + ky] = t
                    # 1/δ in place on δ (done reading: pd consumed it), then
                    # atan args: bc onto fresh bufs, targ = bc*(1/δ) in place
                    for kk in dl:
                        nc.vector.reciprocal_approx_fast(
                            out=dl[kk][:], in_=dl[kk][:])
                    targ = {}
                    for kx in MP:
                        for ky in MP:
                            kk = kx + ky
                            t = wtile("bc")
                            eng(ENG["bc"]).tensor_tensor(
                                t[:], bs[kx][:], cs_[ky][:], OP.mult)
                            eng(ENG["targ"]).tensor_tensor(
                                t[:], t[:], dl[kk][:], OP.mult)
                            targ[kk] = t
                    e.update(prod=prod, targ=targ)

                def st4(i):
                    """ACT ln phase: lax = Ln(sq+1) onto sq, lnp = Ln(prod)
                    onto prod."""
                    e = st[i]
                    lax, lnp = {}, {}
                    for sd, sq in (("b", e["sqb"]), ("c", e["sqc"])):
                        for k in MP:
                            t = sq[k]
                            _act(t[:], t[:], AF.Ln, bias=1.0)
                            lax[sd + k] = t
                    for sk, t in e["prod"].items():
                        _act(t[:], t[:], AF.Ln)
                        lnp[sk] = t
                    e.update(lax=lax, lnp=lnp)

                def st5(i):
                    """lnq = lax - lnp; blnq = b'*lnq; accumulate with
                    diag(-s_i)."""
                    e = st[i]
                    for sd, op in (("b", e["b"]), ("c", e["c"])):
                        for k in MP:
                            t = e["lax"][sd + k]
                            eng(ENG["sub"]).tensor_tensor(
                                t[:], t[:], e["lnp"][sd + k][:], OP.subtract)
                            eng(ENG["blnq"]).tensor_tensor(
                                t[:], op[k][:], t[:], OP.mult)
                            accum(t, i)

                def st6(i):
                    """ACT atan phase, in place on targ."""
                    e = st[i]
                    for kk, t in e["targ"].items():
                        _act(t[:], t[:], AF.Arctan)

                def st7(i):
                    """Accumulate the 4 atan tiles with diag(-s_i)."""
                    e = st[i]
                    for kk, t in e["targ"].items():
                        accum(t, i)
                    st[i] = {}   # drop tile refs

                stages = [st0, st1, st2, st3, st4, st5, st6, st7]
                for step, i in SCHEDULE:
                    stages[step](i)
                # evict PSUM -> SBUF (+endC) on ACT (Identity is in every
                # table -> no table load), then DMA out
                for h in range(W // HALF):
                    _act(res[:, h * HALF:(h + 1) * HALF], acc[h][:],
                         AF.Identity, bias=endC)
                nc.sync.dma_start(out[:, cs], res[:])
    nc.finalize()
    return nc


def _host_params(cx, cy, w, h, Pw, A, a, B_off, lx, ly, rows):
    """Per-core [128, NPAR] parameter matrix (per-partition scalars)."""
    pr = np.zeros((P, NPAR), dtype=np.float32)
    for i in range(NCHIP):
        la = a * lx[i]
        ha = a * ly[i]
        W0 = 0.5 * w[rows, i] / la
        H0 = 0.5 * h[rows, i] / ha
        cxl = cx[rows, i] / la
        cyl = cy[rows, i] / ha
        pr[:, 4 * i + 0] = np.repeat(W0 + cxl, REP)   # b'm = -x/(a lx) + .
        pr[:, 4 * i + 1] = np.repeat(W0 - cxl, REP)   # b'p = +x/(a lx) + .
        pr[:, 4 * i + 2] = np.repeat(H0 + cyl, REP)
        pr[:, 4 * i + 3] = np.repeat(H0 - cyl, REP)
    pr[:, 4 * NCHIP] = np.repeat(A * B_off * Pw[rows].sum(axis=1), REP)
    return np.ascontiguousarray(pr, dtype=np.float32)


def _host_diag(Pw, A, a, rows):
    """[128, NCHIP*128] fp32: per chiplet a diag(-Pi*A*a*2/sqrt(pi))."""
    dg = np.zeros((P, NCHIP * P), dtype=np.float32)
    idx = np.arange(P)
    for i in range(NCHIP):
        s = np.repeat(-C1 * A * a * Pw[rows, i], REP).astype(np.float32)
        dg[idx, i * P + idx] = s
    return np.ascontiguousarray(dg)


_CACHE = {}


def _get_executor(scal):
    """Build (once) the Bass program and a cached jitted SPMD callable.

    The callable takes the device-resident inputs as [8·128, ...] arrays
    sharded over 8 cores; the output scratch buffer is created on-device
    inside the same jit call (no host transfer)."""
    if "exec" in _CACHE:
        return _CACHE["exec"]

    import jax
    import jax.numpy as jnp
    from jax.sharding import Mesh, NamedSharding, PartitionSpec
    from jax.experimental.shard_map import shard_map
    from concourse import bass2jax
    import concourse.mybir as mybir

    nc = _build_program(scal)
    _CACHE["nc"] = nc
    bass2jax.install_neuronx_cc_hook()

    partition_name = (nc.partition_id_tensor.name
                      if nc.partition_id_tensor else None)
    in_names, out_names, out_avals = [], [], []
    for alloc in nc.m.functions[0].allocations:
        if not isinstance(alloc, mybir.MemoryLocationSet):
            continue
        name = alloc.memorylocations[0].name
        if alloc.kind == "ExternalInput":
            if name != partition_name:
                in_names.append(name)
        elif alloc.kind == "ExternalOutput":
            out_names.append(name)
            out_avals.append(jax.core.ShapedArray(
                tuple(alloc.tensor_shape), mybir.dt.np(alloc.dtype)))
    n_params = len(in_names)
    all_names = in_names + out_names
    if partition_name is not None:
        all_names = all_names + [partition_name]

    def _body(*args):
        # args = real inputs + one dummy buffer per output. On this
        # (axon/PJRT) path the output operands are inert: the NEFF rename
        # binds the bass "out" tensor to the custom-call RESULT buffer, so
        # the dummy is never read -- it only satisfies the hook's
        # param-order check. One persistent buffer is reused every call.
        operands = list(args)
        if partition_name is not None:
            operands.append(bass2jax.partition_id_tensor())
        outs = bass2jax._bass_exec_p.bind(
            *operands,
            out_avals=tuple(out_avals),
            in_names=tuple(all_names),
            out_names=tuple(out_names),
            lowering_input_output_aliases=(),
            sim_require_finite=True,
            sim_require_nnan=True,
            nc=nc,
        )
        return tuple(outs)

    devices = jax.devices()[:N_CORES]
    mesh = Mesh(np.asarray(devices), ("core",))
    sharding = NamedSharding(mesh, PartitionSpec("core"))
    sharded = jax.jit(
        shard_map(_body, mesh=mesh,
                  in_specs=(PartitionSpec("core"),) * (n_params + len(out_avals)),
                  out_specs=(PartitionSpec("core"),) * len(out_avals),
                  check_rep=False),
        keep_unused=True)

    # device-side dummy output operands (content never read)
    zshapes = [(N_CORES * s.shape[0], *s.shape[1:]) for s in out_avals]
    zdtypes = [s.dtype for s in out_avals]

    def _mk(shape_dtype):
        shape, dtype = shape_dtype
        return jax.jit(lambda: jnp.zeros(shape, dtype),
                       out_shardings=sharding)

    zeros_fns = [_mk(sd) for sd in zip(zshapes, zdtypes)]
    ex = {"sharded": sharded, "in_names": in_names, "zeros_fns": zeros_fns,
          "sharding": sharding, "n_params": n_params}
    _CACHE["exec"] = ex
    return ex


def _scal_from_inputs(a, lx, ly):
    af = float(np.asarray(a).reshape(-1)[0])
    lxf = np.asarray(lx, dtype=np.float64)
    lyf = np.asarray(ly, dtype=np.float64)
    return {
        "inv_la": [float(1.0 / (af * lxf[i])) for i in range(NCHIP)],
        "inv_ha": [float(1.0 / (af * lyf[i])) for i in range(NCHIP)],
    }


def _device_inputs(x, y, chiplets_x, chiplets_y, chiplets_width,
                   chiplets_height, chiplets_power, A, a, B_off, lx, ly):
    """Full-input -> per-core-stacked device arrays keyed by tensor name."""
    x = np.asarray(x, dtype=np.float32)
    y = np.asarray(y, dtype=np.float32)
    cx = np.asarray(chiplets_x, dtype=np.float32)
    cy = np.asarray(chiplets_y, dtype=np.float32)
    w = np.asarray(chiplets_width, dtype=np.float32)
    h = np.asarray(chiplets_height, dtype=np.float32)
    Pw = np.asarray(chiplets_power, dtype=np.float32)
    Af = float(np.asarray(A).reshape(-1)[0])
    af = float(np.asarray(a).reshape(-1)[0])
    Bf = float(np.asarray(B_off).reshape(-1)[0])
    lxf = np.asarray(lx, dtype=np.float64)
    lyf = np.asarray(ly, dtype=np.float64)

    xs = np.ascontiguousarray(x.reshape(N_CORES * P, F))
    ys = np.ascontiguousarray(y.reshape(N_CORES * P, F))
    prs = np.concatenate(
        [_host_params(cx, cy, w, h, Pw, Af, af, Bf, lxf, lyf,
                      slice(c * RPC, (c + 1) * RPC)) for c in range(N_CORES)],
        axis=0)
    dgs = np.concatenate(
        [_host_diag(Pw, Af, af, slice(c * RPC, (c + 1) * RPC))
         for c in range(N_CORES)], axis=0)
    return {"xin": xs, "yin": ys, "prm": prs, "dgf": dgs}


def run(x, y, chiplets_x, chiplets_y, chiplets_width, chiplets_height,
        chiplets_power, A, a, B_off, lx, ly, grid=None):
    import jax

    ex = _get_executor(_scal_from_inputs(a, lx, ly))
    arrs = _device_inputs(x, y, chiplets_x, chiplets_y, chiplets_width,
                          chiplets_height, chiplets_power, A, a, B_off,
                          lx, ly)
    ins = [jax.device_put(arrs[nm], ex["sharding"]) for nm in ex["in_names"]]
    scratch = [zf() for zf in ex["zeros_fns"]]
    out = ex["sharded"](*ins, *scratch)
    full = np.asarray(out[0]).reshape(B, G2).astype(np.float32, copy=False)
    return full


def kernel(**inputs):
    return run(**inputs)


# revision 12
# speedup vs baseline: 70.1551x; 1.3932x over previous
"""Trainium2 Bass kernel for the ChipletThermalModel problem.

Math per chiplet i, per grid point (summed over 16 chiplets), after
normalizing by `a` (F(a,b,c) = a*F(1, b/a, c/a), so a^2 -> 1 and the
overall factor a folds into the per-chiplet scale):
  b'± = pb± ± x/(a*lx),  c'± = pc± ± y/(a*ly)       (pb,pc host-precomputed)
  For the 4 sign combos k=(s1,s2):
    rδ_k = AbsRsqrt(s0_k + 1) = 1/δ_k   (ACT table op, ~4e-5 rel err)
    δ_k  = (s0_k + 1) * rδ_k            (one DVE STT)
  b-side (pair-merged over s2):
    lnq_b(s1) = ln(1+b'²) - ln((c'm+δ_{s1m})(c'p+δ_{s1p}))
    contribution -= s_i * b'_{s1} * lnq_b(s1)        (s_i = Pi*A*a*2/√π)
  c-side symmetric; atan side: contribution -= s_i * atan(b'c' * rδ_k)
  (the HW ISA has no tensor-tensor divide; recip-approx custom DVE ops
  measure 3.1us/op, AbsRsqrt on ACT is far cheaper) plus endC=A*B_off*ΣPi.

The -s_i scale is applied by the accumulating PE matmul with a diagonal
stationary matrix diag(-s_i[p]) (host-precomputed per partition), so no
elementwise scale ops remain.

Engine split per [128,1024] fp32 tile per chiplet, using MEASURED HW
throughputs (Pool mult 5.8us -> Pool gets only add/sub; DVE TT 1.20us,
DVE TS 1.00us, Pool add ~2.0us, ACT ~1.43us):
  ACT : 18 ops in 3 table phases (4 AbsRsqrt | 4+4 Ln | 4 Atan) + 2
        Square (in every table -> no extra load) + PSUM eviction.
  DVE : 4 prep tensor_scalar + 2 sq + 4 δ-STT + 16 mult TT.
  Pool: 16 add/sub TT.
  PE  : 16 matmul-halves [128,512] fp32 accumulating into PSUM.

Sharding: batch dim (64) split across 8 cores -> 8 rows/core, laid out as
[128 partitions, 4096]; per-(batch-row,chiplet) parameters become
per-partition [128,1] scalars and [128,128] diag blocks (host-precomputed).
"""
import sys
import numpy as np

for _p in ("/opt/trn_rl_repo",):
    if _p not in sys.path:
        sys.path.insert(0, _p)

N_CORES = 8
B, NCHIP, G2 = 64, 16, 65536
RPC = B // N_CORES            # batch rows per core = 8
P = 128                       # SBUF partitions
F = RPC * G2 // P             # free-dim columns per core = 4096
W = 1024                      # columns per processing group
NG = F // W                   # groups
WK_BUFS = 40                  # work-tile ring size
REP = P // RPC                # partitions per batch row = 16
NPAR = 4 * NCHIP + 1          # params columns (4 per chiplet + endC)
C1 = float(2.0 / np.sqrt(np.pi))

# Which of the flexible elementwise ops run on DVE ("d") vs Pool ("p") vs
# ACT ("a", unary-capable ops only). Lists give per-index picks.
# Measured HW: Pool can only afford adds/subs; all mults go to DVE/ACT.
ENG = {
    "sq": ["d", "d", "a", "a"],          # b'm², b'p², c'm², c'p²
    "s0": "p",
    "pd": ["p", "p", "p", "p", "p", "p", "p", "p"],
    "prod": "d", "sub": "p", "bc": "d", "targ": "d", "blnq": "d",
}


def _make_schedule(mode):
    """Emission order over (stage, chiplet). Engines dispatch in order with
    head-of-line blocking; pair-blocked ACT phases keep table loads at 3
    per chiplet pair, and hoisting the next pair's early stages keeps the
    vector engines fed while ACT chews a phase."""
    s = []
    if mode == "pairs":
        for pr in range(NCHIP // 2):
            i0, i1 = 2 * pr, 2 * pr + 1
            s += [(0, i0), (1, i0), (0, i1), (1, i1),
                  (2, i0), (2, i1), (3, i0), (3, i1),
                  (4, i0), (4, i1), (5, i0), (5, i1),
                  (6, i0), (6, i1), (7, i0), (7, i1)]
    elif mode == "sw":
        # software-pipelined: next pair's prep/squares emitted between this
        # pair's ln phase and blnq stage
        s += [(0, 0), (1, 0), (0, 1), (1, 1)]
        for pr in range(NCHIP // 2):
            i0, i1 = 2 * pr, 2 * pr + 1
            j0, j1 = i0 + 2, i1 + 2
            s += [(2, i0), (2, i1), (3, i0), (3, i1),
                  (4, i0), (4, i1),
                  (0, j0), (1, j0), (0, j1), (1, j1),
                  (5, i0), (5, i1),
                  (6, i0), (6, i1), (7, i0), (7, i1)]
    return [t for t in s if 0 <= t[1] < NCHIP]


SCHEDULE = _make_schedule("sw")


def _build_program(scal):
    """Build the Bass program. `scal` holds python-float per-chiplet scalars."""
    from concourse import bacc, tile
    import concourse.mybir as mybir
    import bass_rust as _bass_rust

    AF = mybir.ActivationFunctionType
    OP = mybir.AluOpType
    FP32 = mybir.dt.float32

    nc = bacc.Bacc("TRN2", target_bir_lowering=False, debug=False,
                   enable_asserts=False)

    # Pin the ACT instruction order with scheduler-only (nosync) dep edges:
    # the engine is in-order so same-engine ordering costs nothing at
    # runtime, but it stops the list scheduler from interleaving ops of
    # different table phases (table thrash).
    _act_prev = [None]

    def _act(out, in_, func, **kw):
        inst = nc.scalar.activation(out, in_, func, **kw)
        if _act_prev[0] is not None:
            _bass_rust.add_dep_helper(inst.ins, _act_prev[0], sync=False,
                                      reason="act table phase order")
        _act_prev[0] = inst.ins
        return inst

    xin = nc.dram_tensor("xin", [P, F], FP32, kind="ExternalInput")
    yin = nc.dram_tensor("yin", [P, F], FP32, kind="ExternalInput")
    prm = nc.dram_tensor("prm", [P, NPAR], FP32, kind="ExternalInput")
    dgf = nc.dram_tensor("dgf", [P, NCHIP * P], FP32, kind="ExternalInput")
    out = nc.dram_tensor("out", [P, F], FP32, kind="ExternalOutput")

    inv_la = scal["inv_la"]
    inv_ha = scal["inv_ha"]

    MP = ("m", "p")
    HALF = W // 2

    def eng(which):
        return nc.vector if which == "d" else nc.gpsimd

    with tile.TileContext(nc) as tc:
        with tc.tile_pool(name="cst", bufs=1) as cst, \
             tc.tile_pool(name="io", bufs=2) as io, \
             tc.tile_pool(name="ps", bufs=2, space="PSUM") as ps, \
             tc.tile_pool(name="wk", bufs=WK_BUFS) as wk:
            prmt = cst.tile([P, NPAR], FP32)
            nc.sync.dma_start(prmt[:], prm[:])
            dgt = cst.tile([P, NCHIP * P], FP32)
            nc.sync.dma_start(dgt[:], dgf[:])

            def pcol(i, k):           # [128,1] per-partition param AP
                return prmt[:, 4 * i + k: 4 * i + k + 1]

            endC = prmt[:, 4 * NCHIP: 4 * NCHIP + 1]

            for g in range(NG):
                cs = slice(g * W, (g + 1) * W)
                xt = io.tile([P, W], FP32, tag="xt")
                yt = io.tile([P, W], FP32, tag="yt")
                res = io.tile([P, W], FP32, tag="res")
                nc.sync.dma_start(xt[:], xin[:, cs])
                nc.sync.dma_start(yt[:], yin[:, cs])
                # PSUM accumulators: one bank per half-tile (matmul moving
                # free dim is capped at 512)
                acc = [ps.tile([P, HALF], FP32, tag=f"acc{h}",
                               name=f"acc{h}")
                       for h in range(W // HALF)]
                mm_count = [0]
                MM_TOTAL = NCHIP * 8 * (W // HALF)

                def accum(t, i):
                    """res_psum += diag(-s_i) @ t on the PE."""
                    stat = dgt[:, i * P:(i + 1) * P]
                    for h, a_ in enumerate(acc):
                        first = mm_count[0] < len(acc)
                        last = mm_count[0] >= MM_TOTAL - len(acc)
                        nc.tensor.matmul(
                            a_[:], stat, t[:, h * HALF:(h + 1) * HALF],
                            start=first, stop=last)
                        mm_count[0] += 1

                def wtile(nm):
                    return wk.tile([P, W], FP32, tag="wk", name=nm)

                # per-chiplet state dicts, keyed by chiplet index
                st = [dict() for _ in range(NCHIP)]

                def st0(i):
                    """b'±, c'± via DVE tensor_scalar (x*(±1/(a*l)) + p)."""
                    e = st[i]
                    bs, cs_ = {}, {}
                    for k, sgn, col in (("m", -1.0, 0), ("p", 1.0, 1)):
                        t = wtile("b" + k)
                        nc.vector.tensor_scalar(t[:], xt[:], sgn * inv_la[i],
                                                pcol(i, col), OP.mult, OP.add)
                        bs[k] = t
                    for k, sgn, col in (("m", -1.0, 2), ("p", 1.0, 3)):
                        t = wtile("c" + k)
                        nc.vector.tensor_scalar(t[:], yt[:], sgn * inv_ha[i],
                                                pcol(i, col), OP.mult, OP.add)
                        cs_[k] = t
                    e.update(b=bs, c=cs_)

                def st1(i):
                    """squares and s0 = b'² + c'² per combo."""
                    e = st[i]
                    sqb, sqc = {}, {}
                    srcs = [("b", e["b"]), ("c", e["c"])]
                    sqi = 0
                    for nm, src in srcs:
                        d = sqb if nm == "b" else sqc
                        for k in MP:
                            t = wtile("sq" + nm)
                            which = ENG["sq"][sqi]
                            sqi += 1
                            if which == "a":
                                _act(t[:], src[k][:], AF.Square)
                            else:
                                eng(which).tensor_tensor(
                                    t[:], src[k][:], src[k][:], OP.mult)
                            d[k] = t
                    s0 = {}
                    for kx in MP:
                        for ky in MP:
                            t = wtile("s0")
                            eng(ENG["s0"]).tensor_tensor(
                                t[:], sqb[kx][:], sqc[ky][:], OP.add)
                            s0[kx + ky] = t
                    e.update(sqb=sqb, sqc=sqc, s0=s0)

                def st2(i):
                    """ACT rsqrt phase: rδ = AbsRsqrt(s0+1); δ = (s0+1)*rδ
                    (DVE STT, in place onto s0)."""
                    e = st[i]
                    rd, dl = {}, {}
                    for kk in ("mm", "mp", "pm", "pp"):
                        t = wtile("rd")
                        _act(t[:], e["s0"][kk][:], AF.Abs_reciprocal_sqrt,
                             bias=1.0)
                        rd[kk] = t
                    for kk in ("mm", "mp", "pm", "pp"):
                        t = e["s0"][kk]      # in place: (s0+1)*rδ -> δ
                        nc.vector.scalar_tensor_tensor(
                            t[:], t[:], 1.0, rd[kk][:], OP.add, OP.mult)
                        dl[kk] = t
                    e.update(rd=rd, dl=dl)

                def st3(i):
                    """pd sums, pair products, bc, targ = bc*rδ (DVE+Pool)."""
                    e = st[i]
                    bs, cs_, dl, rd = e["b"], e["c"], e["dl"], e["rd"]
                    cpd, bpd = {}, {}
                    pdi = 0
                    for kx in MP:
                        for ky in MP:
                            kk = kx + ky
                            t = wtile("cpd")
                            eng(ENG["pd"][pdi]).tensor_tensor(
                                t[:], cs_[ky][:], dl[kk][:], OP.add)
                            cpd[kk] = t
                            pdi += 1
                    for kx in MP:
                        for ky in MP:
                            kk = kx + ky
                            t = dl[kk]           # δ dead after bpd
                            eng(ENG["pd"][pdi]).tensor_tensor(
                                t[:], bs[kx][:], dl[kk][:], OP.add)
                            bpd[kk] = t
                            pdi += 1
                    # pair products; write onto one of the inputs
                    prod = {}
                    for kx in MP:      # b-side: fixed s1, product over s2
                        t = cpd[kx + "m"]
                        eng(ENG["prod"]).tensor_tensor(
                            t[:], cpd[kx + "m"][:], cpd[kx + "p"][:],
                            OP.mult)
                        prod["b" + kx] = t
                    for ky in MP:      # c-side: fixed s2, product over s1
                        t = bpd["m" + ky]
                        eng(ENG["prod"]).tensor_tensor(
                            t[:], bpd["m" + ky][:], bpd["p" + ky][:],
                            OP.mult)
                        prod["c" + ky] = t
                    # atan args: bc fresh, targ = bc*rδ in place on bc
                    targ = {}
                    for kx in MP:
                        for ky in MP:
                            kk = kx + ky
                            t = wtile("bc")
                            eng(ENG["bc"]).tensor_tensor(
                                t[:], bs[kx][:], cs_[ky][:], OP.mult)
                            eng(ENG["targ"]).tensor_tensor(
                                t[:], t[:], rd[kk][:], OP.mult)
                            targ[kk] = t
                    e.update(prod=prod, targ=targ)

                def st4(i):
                    """ACT ln phase: lax = Ln(sq+1) onto sq, lnp = Ln(prod)
                    onto prod."""
                    e = st[i]
                    lax, lnp = {}, {}
                    for sd, sq in (("b", e["sqb"]), ("c", e["sqc"])):
                        for k in MP:
                            t = sq[k]
                            _act(t[:], t[:], AF.Ln, bias=1.0)
                            lax[sd + k] = t
                    for sk, t in e["prod"].items():
                        _act(t[:], t[:], AF.Ln)
                        lnp[sk] = t
                    e.update(lax=lax, lnp=lnp)

                def st5(i):
                    """lnq = lax - lnp; blnq = b'*lnq; accumulate with
                    diag(-s_i)."""
                    e = st[i]
                    for sd, op in (("b", e["b"]), ("c", e["c"])):
                        for k in MP:
                            t = e["lax"][sd + k]
                            eng(ENG["sub"]).tensor_tensor(
                                t[:], t[:], e["lnp"][sd + k][:], OP.subtract)
                            eng(ENG["blnq"]).tensor_tensor(
                                t[:], op[k][:], t[:], OP.mult)
                            accum(t, i)

                def st6(i):
                    """ACT atan phase, in place on targ."""
                    e = st[i]
                    for kk, t in e["targ"].items():
                        _act(t[:], t[:], AF.Arctan)

                def st7(i):
                    """Accumulate the 4 atan tiles with diag(-s_i)."""
                    e = st[i]
                    for kk, t in e["targ"].items():
                        accum(t, i)
                    st[i] = {}   # drop tile refs

                stages = [st0, st1, st2, st3, st4, st5, st6, st7]
                for step, i in SCHEDULE:
                    stages[step](i)
                # evict PSUM -> SBUF (+endC) on ACT (Identity is in every
                # table -> no table load), then DMA out
                for h in range(W // HALF):
                    _act(res[:, h * HALF:(h + 1) * HALF], acc[h][:],
                         AF.Identity, bias=endC)
                nc.sync.dma_start(out[:, cs], res[:])
    nc.finalize()
    return nc


def _host_params(cx, cy, w, h, Pw, A, a, B_off, lx, ly, rows):
    """Per-core [128, NPAR] parameter matrix (per-partition scalars)."""
    pr = np.zeros((P, NPAR), dtype=np.float32)
    for i in range(NCHIP):
        la = a * lx[i]
        ha = a * ly[i]
        W0 = 0.5 * w[rows, i] / la
        H0 = 0.5 * h[rows, i] / ha
        cxl = cx[rows, i] / la
        cyl = cy[rows, i] / ha
        pr[:, 4 * i + 0] = np.repeat(W0 + cxl, REP)   # b'm = -x/(a lx) + .
        pr[:, 4 * i + 1] = np.repeat(W0 - cxl, REP)   # b'p = +x/(a lx) + .
        pr[:, 4 * i + 2] = np.repeat(H0 + cyl, REP)
        pr[:, 4 * i + 3] = np.repeat(H0 - cyl, REP)
    pr[:, 4 * NCHIP] = np.repeat(A * B_off * Pw[rows].sum(axis=1), REP)
    return np.ascontiguousarray(pr, dtype=np.float32)


def _host_diag(Pw, A, a, rows):
    """[128, NCHIP*128] fp32: per chiplet a diag(-Pi*A*a*2/sqrt(pi))."""
    dg = np.zeros((P, NCHIP * P), dtype=np.float32)
    idx = np.arange(P)
    for i in range(NCHIP):
        s = np.repeat(-C1 * A * a * Pw[rows, i], REP).astype(np.float32)
        dg[idx, i * P + idx] = s
    return np.ascontiguousarray(dg)


_CACHE = {}


def _get_executor(scal):
    """Build (once) the Bass program and a cached jitted SPMD callable.

    The callable takes the device-resident inputs as [8·128, ...] arrays
    sharded over 8 cores; the output scratch buffer is created on-device
    inside the same jit call (no host transfer)."""
    if "exec" in _CACHE:
        return _CACHE["exec"]

    import jax
    import jax.numpy as jnp
    from jax.sharding import Mesh, NamedSharding, PartitionSpec
    from jax.experimental.shard_map import shard_map
    from concourse import bass2jax
    import concourse.mybir as mybir

    nc = _build_program(scal)
    _CACHE["nc"] = nc
    bass2jax.install_neuronx_cc_hook()

    partition_name = (nc.partition_id_tensor.name
                      if nc.partition_id_tensor else None)
    in_names, out_names, out_avals = [], [], []
    for alloc in nc.m.functions[0].allocations:
        if not isinstance(alloc, mybir.MemoryLocationSet):
            continue
        name = alloc.memorylocations[0].name
        if alloc.kind == "ExternalInput":
            if name != partition_name:
                in_names.append(name)
        elif alloc.kind == "ExternalOutput":
            out_names.append(name)
            out_avals.append(jax.core.ShapedArray(
                tuple(alloc.tensor_shape), mybir.dt.np(alloc.dtype)))
    n_params = len(in_names)
    all_names = in_names + out_names
    if partition_name is not None:
        all_names = all_names + [partition_name]

    def _body(*args):
        # args = real inputs + one dummy buffer per output. On this
        # (axon/PJRT) path the output operands are inert: the NEFF rename
        # binds the bass "out" tensor to the custom-call RESULT buffer, so
        # the dummy is never read -- it only satisfies the hook's
        # param-order check. One persistent buffer is reused every call.
        operands = list(args)
        if partition_name is not None:
            operands.append(bass2jax.partition_id_tensor())
        outs = bass2jax._bass_exec_p.bind(
            *operands,
            out_avals=tuple(out_avals),
            in_names=tuple(all_names),
            out_names=tuple(out_names),
            lowering_input_output_aliases=(),
            sim_require_finite=True,
            sim_require_nnan=True,
            nc=nc,
        )
        return tuple(outs)

    devices = jax.devices()[:N_CORES]
    mesh = Mesh(np.asarray(devices), ("core",))
    sharding = NamedSharding(mesh, PartitionSpec("core"))
    sharded = jax.jit(
        shard_map(_body, mesh=mesh,
                  in_specs=(PartitionSpec("core"),) * (n_params + len(out_avals)),
                  out_specs=(PartitionSpec("core"),) * len(out_avals),
                  check_rep=False),
        keep_unused=True)

    # device-side dummy output operands (content never read)
    zshapes = [(N_CORES * s.shape[0], *s.shape[1:]) for s in out_avals]
    zdtypes = [s.dtype for s in out_avals]

    def _mk(shape_dtype):
        shape, dtype = shape_dtype
        return jax.jit(lambda: jnp.zeros(shape, dtype),
                       out_shardings=sharding)

    zeros_fns = [_mk(sd) for sd in zip(zshapes, zdtypes)]
    ex = {"sharded": sharded, "in_names": in_names, "zeros_fns": zeros_fns,
          "sharding": sharding, "n_params": n_params}
    _CACHE["exec"] = ex
    return ex


def _scal_from_inputs(a, lx, ly):
    af = float(np.asarray(a).reshape(-1)[0])
    lxf = np.asarray(lx, dtype=np.float64)
    lyf = np.asarray(ly, dtype=np.float64)
    return {
        "inv_la": [float(1.0 / (af * lxf[i])) for i in range(NCHIP)],
        "inv_ha": [float(1.0 / (af * lyf[i])) for i in range(NCHIP)],
    }


def _device_inputs(x, y, chiplets_x, chiplets_y, chiplets_width,
                   chiplets_height, chiplets_power, A, a, B_off, lx, ly):
    """Full-input -> per-core-stacked device arrays keyed by tensor name."""
    x = np.asarray(x, dtype=np.float32)
    y = np.asarray(y, dtype=np.float32)
    cx = np.asarray(chiplets_x, dtype=np.float32)
    cy = np.asarray(chiplets_y, dtype=np.float32)
    w = np.asarray(chiplets_width, dtype=np.float32)
    h = np.asarray(chiplets_height, dtype=np.float32)
    Pw = np.asarray(chiplets_power, dtype=np.float32)
    Af = float(np.asarray(A).reshape(-1)[0])
    af = float(np.asarray(a).reshape(-1)[0])
    Bf = float(np.asarray(B_off).reshape(-1)[0])
    lxf = np.asarray(lx, dtype=np.float64)
    lyf = np.asarray(ly, dtype=np.float64)

    xs = np.ascontiguousarray(x.reshape(N_CORES * P, F))
    ys = np.ascontiguousarray(y.reshape(N_CORES * P, F))
    prs = np.concatenate(
        [_host_params(cx, cy, w, h, Pw, Af, af, Bf, lxf, lyf,
                      slice(c * RPC, (c + 1) * RPC)) for c in range(N_CORES)],
        axis=0)
    dgs = np.concatenate(
        [_host_diag(Pw, Af, af, slice(c * RPC, (c + 1) * RPC))
         for c in range(N_CORES)], axis=0)
    return {"xin": xs, "yin": ys, "prm": prs, "dgf": dgs}


def run(x, y, chiplets_x, chiplets_y, chiplets_width, chiplets_height,
        chiplets_power, A, a, B_off, lx, ly, grid=None):
    import jax

    ex = _get_executor(_scal_from_inputs(a, lx, ly))
    arrs = _device_inputs(x, y, chiplets_x, chiplets_y, chiplets_width,
                          chiplets_height, chiplets_power, A, a, B_off,
                          lx, ly)
    ins = [jax.device_put(arrs[nm], ex["sharding"]) for nm in ex["in_names"]]
    scratch = [zf() for zf in ex["zeros_fns"]]
    out = ex["sharded"](*ins, *scratch)
    full = np.asarray(out[0]).reshape(B, G2).astype(np.float32, copy=False)
    return full


def kernel(**inputs):
    return run(**inputs)
